# revision 1
# baseline (speedup 1.0000x reference)
"""Multi-head attention (MemoryNet) Bass kernel for 8 Trainium2 cores.

Problem (per reference):
  q,k: [b=4, d=1024, m/n=2048], v: [4, 1024, 2048] fp32, N_HEAD=8
  per head (32 total): S = (qh^T kh)/sqrt(128); P = softmax(S, axis=-1)
  out_head = vh @ P^T  -> [128, 2048]; out = [4, 1024, 2048]

Sharding: 32 heads = 8 cores x 4 heads; pure head parallelism.

Per-core schedule (per head, software-pipelined across heads):
  - load q,k,v head slices fp32, cast bf16 (emitted one head ahead)
  - vT via ONE DMA-xbar transpose/head + strided DVE copy into the packed
    [vT | ones] layout (ones column -> softmax denominator Z)
  - chunk loop j=0..15:
      S^T[n_j, m] = k_j^T q on TensorE (fp32 PSUM, 2 half-tiles)
      exp(scale*S^T) on ScalarE PSUM->SBUF bf16 (no max subtraction:
        scores bounded, softmax shift-invariant; ACT is the kernel's
        critical path at ~2.3us/chunk)
      pass-A AV (m-tiles 0-2, one psum bank each) consumes chunk j
        immediately: O^T[m,132] += expS^T_j.T @ [vT_j | 1]
      the other 13 m-tiles run as AV slices of the PREVIOUS head,
        spread over chunks j=1..10 (t-sequential within each shared bank)
  - finish: 1/Z via DVE reciprocal, per-partition scale (fp32) into the
    out buffer; store O^T[m, c] via SWDGE. Host un-transposes per head
    during the gather.
"""

import sys

sys.path.insert(0, "/opt/trn_rl_repo")

import numpy as np

N_CORES = 8
HPC = 4  # heads per core
DH = 128  # head dim (contraction for QK)
M = 2048  # queries
NK = 2048  # keys
CH = 128  # v channels per head
NT = NK // 128  # 16 n-chunks
MT = M // 128  # 16 m-tiles
SCALE = 1.0 / float(np.sqrt(DH))

# Pass-A m-tiles: consumed chunk-by-chunk, each in its OWN psum bank
# (hw: a matmul with start=True resets accumulation state for the whole
# bank, so start-groups must never interleave within one bank).
A_TILES = [0, 1, 2]
# Remaining m-tiles run as t-sequential pack slices of the PREVIOUS head,
# interleaved into the next head's chunk loop at these chunks.
OLD_PACKS = [[3, 4, 5], [6, 7, 8], [9, 10, 11], [12, 13, 14], [15]]
# chunk j -> list of (pack, t) slice units (each = 16 matmuls + finish)
OLD_AT = {
    1: [(0, 0), (0, 1)], 2: [(0, 2)],
    3: [(1, 0)], 4: [(1, 1)], 5: [(1, 2)],
    6: [(2, 0)], 7: [(2, 1)], 8: [(2, 2)],
    9: [(3, 0), (3, 1)], 10: [(3, 2), (4, 0)],
}

_CACHE = {}


def _build(loop_reps=1):
    from contextlib import ExitStack

    from concourse import bacc, mybir, tile

    f32 = mybir.dt.float32
    bf16 = mybir.dt.bfloat16

    nc = bacc.Bacc("TRN2", target_bir_lowering=False, debug=False,
                   num_devices=N_CORES)
    q4 = nc.dram_tensor("q4", (HPC, DH, M), f32, kind="ExternalInput").ap()
    k4 = nc.dram_tensor("k4", (HPC, DH, NK), f32, kind="ExternalInput").ap()
    v4 = nc.dram_tensor("v4", (HPC, CH, NK), f32, kind="ExternalInput").ap()
    # per-head output is O^T [m, c]; host transposes during gather
    o4t = nc.dram_tensor("o4t", (HPC, M, CH), f32, kind="ExternalOutput").ap()

    with tile.TileContext(nc) as tc, ExitStack() as ctx:
        stage = ctx.enter_context(tc.tile_pool(name="stage", bufs=2))
        bfp = ctx.enter_context(tc.tile_pool(name="bfp", bufs=6))
        vtp = ctx.enter_context(tc.tile_pool(name="vtp", bufs=2))
        vtraw = ctx.enter_context(tc.tile_pool(name="vtraw", bufs=2))
        ep = ctx.enter_context(tc.tile_pool(name="ep", bufs=27))
        outp = ctx.enter_context(tc.tile_pool(name="outp", bufs=2))
        smallp = ctx.enter_context(tc.tile_pool(name="smallp", bufs=10))
        pss = ctx.enter_context(tc.tile_pool(name="pss", bufs=2, space="PSUM"))
        psoa = ctx.enter_context(tc.tile_pool(name="psoa", bufs=3,
                                              space="PSUM"))
        pso = ctx.enter_context(tc.tile_pool(name="pso", bufs=1, space="PSUM"))

        if loop_reps > 1:
            # hardware loop for timing runs: repeats the whole computation
            ctx.enter_context(tc.For_i(0, loop_reps, 1))

        def emit_load(h):
            st = {}
            qf = stage.tile([DH, M], f32, tag="stage", name=f"qf{h}")
            kf = stage.tile([DH, NK], f32, tag="stage", name=f"kf{h}")
            st["qb"] = bfp.tile([DH, M], bf16, tag="bf", name=f"qb{h}")
            st["kb"] = bfp.tile([DH, NK], bf16, tag="bf", name=f"kb{h}")
            vf = stage.tile([CH, NK], f32, tag="stage", name=f"vf{h}")
            vb = bfp.tile([CH, NK], bf16, tag="bf", name=f"vb{h}")
            nc.sync.dma_start(out=qf, in_=q4[h])
            nc.sync.dma_start(out=kf, in_=k4[h])
            nc.sync.dma_start(out=vf, in_=v4[h])
            nc.vector.tensor_copy(st["qb"], qf)
            nc.vector.tensor_copy(st["kb"], kf)
            nc.vector.tensor_copy(vb, vf)
            # one xbar transpose for the whole head: vt[p, j, c] = v[c, 128j+p]
            vt_raw = vtraw.tile([128, NT, 128], bf16, tag="vtr",
                                name=f"vtr{h}")
            nc.sync.dma_start_transpose(out=vt_raw, in_=vb)
            vton = vtp.tile([128, NT, 132], bf16, tag="vt", name=f"vton{h}")
            nc.gpsimd.memset(vton, 1.0)
            nc.vector.tensor_copy(vton[:, :, 0:128], vt_raw)
            st["vton"] = vton
            st["h"] = h
            st["expst"] = []
            return st

        def finish_tile(po, t, i, out_t, h):
            rz = smallp.tile([128, 1], f32, tag="rz", name=f"rz{h}_{i}")
            nc.vector.reciprocal(rz, po[:, t, 128:129])
            nc.vector.tensor_scalar_mul(out_t[:, i, :], po[:, t, 0:128], rz)

        def emit_old_unit(old, pi, t):
            """One (pack, slice) unit: 16 accumulating matmuls + finish.
            Slices within a pack share a bank but run strictly
            t-sequentially (never interleaved start-groups)."""
            pack = OLD_PACKS[pi]
            h = old["h"]
            if t == 0:
                old["po_cur"] = pso.tile([128, 3, 132], f32, tag="po",
                                         name=f"po{h}_r{pi}")
            po = old["po_cur"]
            i = pack[t]
            for j in range(NT):
                nc.tensor.matmul(
                    po[:, t, :],
                    old["expst"][j][:, 128 * i:128 * (i + 1)],
                    old["vton"][:, j, :],
                    start=(j == 0),
                    stop=(j == NT - 1),
                )
            finish_tile(po, t, i, old["out_t"], h)

        old = None
        st = emit_load(0)
        for h in range(HPC):
            out_t = outp.tile([128, MT, CH], f32, tag="out", name=f"out{h}")
            st["out_t"] = out_t
            po_a = [
                psoa.tile([128, 132], f32, tag="poa", name=f"po{h}_a{pi}")
                for pi in range(len(A_TILES))
            ]
            for j in range(NT):
                e = ep.tile([128, M], bf16, tag="e", name=f"e{h}_{j}")
                kslice = st["kb"][:, 128 * j:128 * (j + 1)]
                for half in range(2):
                    s = pss.tile([128, 1024], f32, tag="s",
                                 name=f"s{h}_{j}_{half}")
                    for quarter in range(2):
                        mo = 1024 * half + 512 * quarter
                        nc.tensor.matmul(
                            s[:, 512 * quarter:512 * (quarter + 1)],
                            kslice,
                            st["qb"][:, mo:mo + 512],
                            start=True,
                            stop=True,
                        )
                    nc.scalar.activation(
                        e[:, 1024 * half:1024 * (half + 1)],
                        s,
                        mybir.ActivationFunctionType.Exp,
                        scale=SCALE,
                    )
                st["expst"].append(e)
                # pass-A AV on this chunk (PE trails ACT by <= 1 chunk)
                for pi, i in enumerate(A_TILES):
                    nc.tensor.matmul(
                        po_a[pi][:, :],
                        e[:, 128 * i:128 * (i + 1)],
                        st["vton"][:, j, :],
                        start=(j == 0),
                        stop=(j == NT - 1),
                    )
                if old is not None and j in OLD_AT:
                    for pi, t in OLD_AT[j]:
                        emit_old_unit(old, pi, t)
                if j == 4 and h + 1 < HPC:
                    nxt = emit_load(h + 1)
            if old is not None:
                nc.gpsimd.dma_start(out=o4t[old["h"]].rearrange(
                    "(i p) c -> p i c", p=128), in_=old["out_t"])
            for pi, i in enumerate(A_TILES):
                rz = smallp.tile([128, 1], f32, tag="rz", name=f"rza{h}_{i}")
                nc.vector.reciprocal(rz, po_a[pi][:, 128:129])
                nc.vector.tensor_scalar_mul(out_t[:, i, :],
                                            po_a[pi][:, 0:128], rz)
            old = st
            if h + 1 < HPC:
                st = nxt
        # drain last head's leftovers
        for j in sorted(OLD_AT):
            for pi, t in OLD_AT[j]:
                emit_old_unit(old, pi, t)
        nc.gpsimd.dma_start(out=o4t[old["h"]].rearrange(
            "(i p) c -> p i c", p=128), in_=old["out_t"])

    nc.compile()
    return nc


def _get_nc():
    if "nc" not in _CACHE:
        _CACHE["nc"] = _build()
    return _CACHE["nc"]


def kernel(q, k, v):
    from concourse.bass_utils import run_bass_kernel_spmd

    nc = _get_nc()
    b, d, m = q.shape
    qh = np.ascontiguousarray(q.reshape(32, DH, M))
    kh = np.ascontiguousarray(k.reshape(32, DH, NK))
    vh = np.ascontiguousarray(v.reshape(32, CH, NK))
    in_maps = [
        {
            "q4": qh[HPC * c:HPC * (c + 1)],
            "k4": kh[HPC * c:HPC * (c + 1)],
            "v4": vh[HPC * c:HPC * (c + 1)],
        }
        for c in range(N_CORES)
    ]
    res = run_bass_kernel_spmd(nc, in_maps, core_ids=list(range(N_CORES)))
    # o4t is [HPC, M, CH] per core = O^T per head; transpose to [CH, M]
    out_t = np.concatenate(
        [res.results[c]["o4t"] for c in range(N_CORES)], axis=0
    )  # [32, M, CH]
    out = np.ascontiguousarray(out_t.transpose(0, 2, 1))  # [32, CH, M]
    return out.reshape(b, d, m).astype(np.float32)

